# revision 15
# baseline (speedup 1.0000x reference)
"""Bass/Tile TRN2 kernel for nn_Attn_52776558133289 (additive attention).

Reference math (per batch row b) — NOTE the reference's hidden-term einsum
'boh,hg->boh' is elementwise-times-column-sums, NOT a matvec:
    m[s, h]   = enc[b] @ M_w[:, :E].T + hidden[b,0,h]*S2[h] + M_b[h]
                with S2[h] = sum_g M_w[g, E+h]
    scores[s] = tanh(m) @ V_w[0] + V_b          (V_b cancels in softmax)
    scores    = where(mask, -1e30, scores)
    weights   = softmax(scores)                  (B, S)
    weighted  = weights @ enc[b]                 (B, E)

Sharding: data-parallel over B across 8 NeuronCores; M/V weights replicated.

Per-core dataflow (b_shard rows each):
    encT (host-prepped transposed bf16 copy of enc) --DMA--> SBUF
    encT layout: [e_lo=128p, e_tile, s]
    m.T tile = matmul(lhsT=M1T[e,h-tile], rhs=encT chunk)   (PSUM f32)
    tanh     = ACT Tanh(m.T + hb[h])  -> bf16   (hb = hidden*S2 + M_b, on device)
    scores  += matmul(lhsT=VwT[:,j], rhs=tanh)              (1 x 512 PSUM)
    per s-chunk epilogue (overlaps later matmuls):
      exp(s-32) on ACT (|s| <= H*max|Vw| = 32 exactly => fixed shift, no max),
      mask-zero, bf16 cast, partition_broadcast, then per e-tile
      DVE mul+reduce into wacc4[:, i, chunk]
    row tail: totals -> 1/sum, normalize weights row + weighted accumulators.
"""

import numpy as np
import ml_dtypes

import concourse.bass as bass
import concourse.tile as tile
from concourse import mybir, bacc
from concourse.bass_utils import run_bass_kernel_spmd

B, S, E, H = 32, 2048, 1024, 1024
NCORES = 8
BS = B // NCORES

F32 = mybir.dt.float32
BF16 = mybir.dt.bfloat16
U8 = mybir.dt.uint8

CH = 512  # matmul moving free-dim chunk (one PSUM bank of f32)


def build(bs=BS, s=S, e=E, h=H, num_devices=NCORES):
    """Build + compile the per-core program. All cores run the same program."""
    P = 128
    s_t, e_t, h_t, g_t = s // P, e // P, h // P, h // P
    n_ch = s // CH          # score chunks per row
    qn = 4 if s_t % 4 == 0 else 1   # s-tiles per cast DMA

    nc = bacc.Bacc("TRN2", target_bir_lowering=False, debug=False,
                   num_devices=num_devices)

    encT_d = nc.dram_tensor("encT", [bs, e, s], BF16, kind="ExternalInput")
    mask_d = nc.dram_tensor("mask", [bs, s], U8, kind="ExternalInput")
    m1t_d = nc.dram_tensor("m1t", [e, h], BF16, kind="ExternalInput")
    s2t_d = nc.dram_tensor("s2t", [P, h // P], F32, kind="ExternalInput")
    hidt_d = nc.dram_tensor("hidt", [h, bs], F32, kind="ExternalInput")
    mbt_d = nc.dram_tensor("mbt", [P, h_t], F32, kind="ExternalInput")
    vwt_d = nc.dram_tensor("vwt", [P, h_t], BF16, kind="ExternalInput")
    weighted_d = nc.dram_tensor("weighted", [bs, e], F32, kind="ExternalOutput")
    weights_d = nc.dram_tensor("weights", [bs, s], F32, kind="ExternalOutput")

    encT_r = encT_d.ap().rearrange("b (i p) s -> p b i s", p=P)
    # weighted row r viewed as [e_lo=128p, e_tile] for the scatter store
    wout_r = weighted_d.ap().rearrange("b (i p) -> p b i", p=P)

    with tile.TileContext(nc) as tc:
        import contextlib
        with contextlib.ExitStack() as ctx:
            consts = ctx.enter_context(tc.tile_pool(name="consts", bufs=1))
            tr_pool = ctx.enter_context(tc.tile_pool(name="encT", bufs=2))
            tanh_pool = ctx.enter_context(tc.tile_pool(name="tanh", bufs=4))
            row1 = ctx.enter_context(tc.tile_pool(name="row1", bufs=1))
            row2 = ctx.enter_context(tc.tile_pool(name="row2", bufs=2))
            ps_big = ctx.enter_context(tc.tile_pool(name="psb", bufs=4, space="PSUM"))
            ps_sc = ctx.enter_context(tc.tile_pool(name="pssc", bufs=4, space="PSUM"))

            # ---- replicated weights into SBUF (scalar HWDGE ring; transposes
            # and per-row small DMAs ride the sync ring) ----
            m1t_sb = consts.tile([P, e_t, h], BF16)
            m1t_r = m1t_d.ap().rearrange("(t p) h -> p t h", p=P)
            for i in range(e_t):
                nc.scalar.dma_start(out=m1t_sb[:, i, :], in_=m1t_r[:, i, :])
            s2t_sb = consts.tile([P, h_t], F32)
            nc.scalar.dma_start(out=s2t_sb, in_=s2t_d.ap())
            hidt_sb = consts.tile([P, g_t, bs], F32)
            nc.scalar.dma_start(out=hidt_sb, in_=hidt_d.ap().rearrange(
                "(t p) b -> p t b", p=P))
            mbt_sb = consts.tile([P, h_t], F32)
            nc.scalar.dma_start(out=mbt_sb, in_=mbt_d.ap())
            vwt_sb = consts.tile([P, h_t], BF16)
            nc.scalar.dma_start(out=vwt_sb, in_=vwt_d.ap())
            shift = consts.tile([1, 1], F32)
            nc.vector.memset(shift, -32.0)

            # ---- hb[h,b] = hidden[b,h]*S2[h]+M_b[h]  (S2 host-precomputed) ----
            hb_sb = consts.tile([P, h_t, bs], F32)
            for j in range(h_t):
                nc.vector.tensor_scalar(
                    out=hb_sb[:, j, :], in0=hidt_sb[:, j, :],
                    scalar1=s2t_sb[:, j:j + 1], scalar2=mbt_sb[:, j:j + 1],
                    op0=mybir.AluOpType.mult, op1=mybir.AluOpType.add)

            # ---- per batch row ----
            def emit_load(r, nchunk):
                """DMA row r of host-transposed bf16 encT into SBUF, by
                s-chunks so chunk 0 is ready early."""
                encT_sb = tr_pool.tile([P, e_t, s], BF16, tag="encT",
                                       name=f"encT{r}")
                cw = s // nchunk
                for q in range(nchunk):
                    nc.sync.dma_start(
                        out=encT_sb[:, :, q * cw:(q + 1) * cw],
                        in_=encT_r[:, r, :, q * cw:(q + 1) * cw])
                return encT_sb

            encTs = {0: emit_load(0, 2 * n_ch)}
            for r in range(bs):
                # prefetch next row's load chain ahead of this row's epilogues
                if r + 1 < bs:
                    encTs[r + 1] = emit_load(r + 1, 2)
                encT = encTs.pop(r)

                # mask row -> mz = 1 - mask (f32), early (off critical path)
                mrow = row2.tile([1, s], U8, tag="mrow", name=f"mrow{r}")
                nc.sync.dma_start(out=mrow, in_=mask_d.ap()[r:r + 1, :])
                mz = row2.tile([1, s], F32, tag="mz", name=f"mz{r}")
                nc.vector.tensor_scalar(
                    out=mz, in0=mrow, scalar1=-1.0, scalar2=1.0,
                    op0=mybir.AluOpType.mult, op1=mybir.AluOpType.add)

                # per-row accumulators
                exps = row2.tile([1, s], F32, tag="exps", name=f"exps{r}")
                expm = row2.tile([1, s], F32, tag="expm", name=f"expm{r}")
                wacc4 = row2.tile([P, e_t, n_ch], F32, tag="wacc4",
                                  name=f"wacc4{r}")
                tot4 = row2.tile([1, n_ch], F32, tag="tot4", name=f"tot4{r}")

                sc_ps = [ps_sc.tile([1, CH], F32, tag="scps", name=f"sc{r}_{c}")
                         for c in range(n_ch)]
                groups = [list(range(i, min(i + 2, n_ch)))
                          for i in range(0, n_ch, 2)]
                for grp in groups:
                    pending = None
                    for j in range(h_t):
                        mt = {}
                        for c in grp:
                            mt[c] = ps_big.tile([P, CH], F32, tag="bigps",
                                                name=f"mt{r}_{j}_{c}")
                        for i in range(e_t):
                            first, last = (i == 0), (i == e_t - 1)
                            for c in grp:
                                nc.tensor.matmul(
                                    out=mt[c],
                                    lhsT=m1t_sb[:, i, j * P:(j + 1) * P],
                                    rhs=encT[:, i, c * CH:(c + 1) * CH],
                                    start=first, stop=last)
                        th = {}
                        for c in grp:
                            th[c] = tanh_pool.tile([P, CH], BF16, tag="th",
                                                   name=f"th{r}_{j}_{c}")
                            nc.scalar.activation(
                                out=th[c], in_=mt[c],
                                func=mybir.ActivationFunctionType.Tanh,
                                bias=hb_sb[:, j, r:r + 1], scale=1.0)
                        if pending is not None:
                            pj, pth = pending
                            for c, t_sb in pth.items():
                                nc.tensor.matmul(
                                    out=sc_ps[c], lhsT=vwt_sb[:, pj:pj + 1],
                                    rhs=t_sb,
                                    start=(pj == 0), stop=(pj == h_t - 1))
                        pending = (j, th)
                    pj, pth = pending
                    for c, t_sb in pth.items():
                        nc.tensor.matmul(
                            out=sc_ps[c], lhsT=vwt_sb[:, pj:pj + 1], rhs=t_sb,
                            start=(pj == 0), stop=(pj == h_t - 1))

                    # ---- chunk epilogue: exp, mask, broadcast, mul+reduce ----
                    for c in grp:
                        cs = slice(c * CH, (c + 1) * CH)
                        nc.scalar.activation(
                            out=exps[:, cs], in_=sc_ps[c],
                            func=mybir.ActivationFunctionType.Exp,
                            bias=shift[0:1, 0:1], scale=1.0)
                        nc.vector.tensor_mul(expm[:, cs], exps[:, cs], mz[:, cs])
                        nc.vector.tensor_reduce(
                            out=tot4[:, c:c + 1], in_=expm[:, cs],
                            axis=mybir.AxisListType.X, op=mybir.AluOpType.add)
                        wbf = row2.tile([1, CH], BF16, tag="wbf",
                                        name=f"wbf{r}_{c}")
                        nc.vector.tensor_copy(out=wbf, in_=expm[:, cs])
                        wbc = row2.tile([P, CH], BF16, tag="wbc",
                                        name=f"wbc{r}_{c}")
                        nc.gpsimd.partition_broadcast(wbc, wbf)
                        for i in range(e_t):
                            prod = row2.tile([P, CH], BF16, tag="prod",
                                             name=f"prod{r}_{c}_{i}")
                            # split across engines so the serial tail shrinks:
                            # muls alternate DVE/GpSimd, reduces DVE/ACT-accum
                            if i % 2 == 0:
                                nc.vector.tensor_mul(prod, encT[:, i, cs], wbc)
                                nc.scalar.activation(
                                    out=prod, in_=prod,
                                    func=mybir.ActivationFunctionType.Copy,
                                    accum_out=wacc4[:, i, c:c + 1])
                            else:
                                nc.gpsimd.tensor_mul(prod, encT[:, i, cs], wbc)
                                nc.vector.tensor_reduce(
                                    out=wacc4[:, i, c:c + 1], in_=prod,
                                    axis=mybir.AxisListType.X,
                                    op=mybir.AluOpType.add)

                # ---- row tail: totals, normalize, outputs ----
                tot = row1.tile([1, 1], F32, tag="tot", name=f"tot{r}")
                nc.vector.tensor_reduce(
                    out=tot, in_=tot4, axis=mybir.AxisListType.X,
                    op=mybir.AluOpType.add)
                inv = row1.tile([1, 1], F32, tag="inv", name=f"inv{r}")
                nc.vector.reciprocal(inv, tot)

                wrow = row2.tile([1, s], F32, tag="wrow", name=f"wrow{r}")
                nc.vector.tensor_scalar_mul(wrow, expm, inv[0:1, 0:1])
                nc.sync.dma_start(out=weights_d.ap()[r:r + 1, :], in_=wrow)

                invb = row1.tile([P, 1], F32, tag="invb", name=f"invb{r}")
                nc.gpsimd.partition_broadcast(invb, inv)
                wacc = row1.tile([P, e_t], F32, tag="wacc", name=f"wacc{r}")
                nc.vector.tensor_reduce(
                    out=wacc, in_=wacc4, axis=mybir.AxisListType.X,
                    op=mybir.AluOpType.add)
                wfin = row1.tile([P, e_t], F32, tag="wfin", name=f"wfin{r}")
                nc.vector.tensor_scalar_mul(wfin, wacc, invb[:, 0:1])
                nc.sync.dma_start(out=wout_r[:, r, :], in_=wfin)

    nc.compile()
    return nc


_NC_CACHE = {}


def _get_nc():
    if "nc" not in _NC_CACHE:
        _NC_CACHE["nc"] = build()
    return _NC_CACHE["nc"]


def kernel(hidden, encoder_outputs, mask, M_w, M_b, V_w, V_b,
           _trace=False, _return_res=False):
    P = 128
    h_t = H // P
    hidden = np.asarray(hidden, dtype=np.float32)
    enc = np.asarray(encoder_outputs, dtype=np.float32)
    mask_u8 = np.asarray(mask).astype(np.uint8)
    M_w = np.asarray(M_w, dtype=np.float32)
    M_b = np.asarray(M_b, dtype=np.float32)
    V_w = np.asarray(V_w, dtype=np.float32)

    nc = _get_nc()

    m1t = np.ascontiguousarray(M_w[:, :E].T).astype(ml_dtypes.bfloat16)
    s2 = M_w[:, E:].astype(np.float64).sum(axis=0).astype(np.float32)
    s2t = np.ascontiguousarray(s2.reshape(h_t, P).T)
    mbt = np.ascontiguousarray(M_b.reshape(h_t, P).T).astype(np.float32)
    vwt = np.ascontiguousarray(V_w[0].reshape(h_t, P).T).astype(ml_dtypes.bfloat16)

    in_maps = []
    for core in range(NCORES):
        rows = slice(core * BS, (core + 1) * BS)
        hidt = np.ascontiguousarray(hidden[rows, 0, :].T).astype(np.float32)
        encT = np.ascontiguousarray(
            enc[rows].transpose(0, 2, 1)).astype(ml_dtypes.bfloat16)
        in_maps.append({
            "encT": encT,
            "mask": np.ascontiguousarray(mask_u8[rows]),
            "m1t": m1t, "s2t": s2t, "hidt": hidt, "mbt": mbt, "vwt": vwt,
        })

    res = run_bass_kernel_spmd(nc, in_maps, list(range(NCORES)), trace=_trace)

    weighted = np.concatenate(
        [res.results[c]["weighted"] for c in range(NCORES)], axis=0)
    weights = np.concatenate(
        [res.results[c]["weights"] for c in range(NCORES)], axis=0)
    out = (weighted[:, None, :].astype(np.float32),
           weights[:, None, :].astype(np.float32))
    if _return_res:
        return out, res
    return out


# revision 16
# speedup vs baseline: 1.5782x; 1.5782x over previous
"""Bass/Tile TRN2 kernel for nn_Attn_52776558133289 (additive attention).

Reference math (per batch row b) — NOTE the reference's hidden-term einsum
'boh,hg->boh' is elementwise-times-column-sums, NOT a matvec:
    m[s, h]   = enc[b] @ M_w[:, :E].T + hidden[b,0,h]*S2[h] + M_b[h]
                with S2[h] = sum_g M_w[g, E+h]
    scores[s] = tanh(m) @ V_w[0] + V_b          (V_b cancels in softmax)
    scores    = where(mask, -1e30, scores)
    weights   = softmax(scores)                  (B, S)
    weighted  = weights @ enc[b]                 (B, E)

Sharding: data-parallel over B across 8 NeuronCores; M/V weights replicated.

Per-core dataflow (b_shard rows each):
    encT (host-prepped transposed bf16 copy of enc) --DMA--> SBUF
    encT layout: [e_lo=128p, e_tile, s]
    m.T tile = matmul(lhsT=M1T[e,h-tile], rhs=encT chunk)   (PSUM f32)
    tanh     = ACT Tanh(m.T + hb[h])  -> bf16   (hb = hidden*S2 + M_b, on device)
    scores  += matmul(lhsT=VwT[:,j], rhs=tanh)              (1 x 512 PSUM)
    per s-chunk epilogue (overlaps later matmuls):
      exp(s-32) on ACT (|s| <= H*max|Vw| = 32 exactly => fixed shift, no max),
      mask-zero, bf16 cast, partition_broadcast, then per e-tile
      DVE mul+reduce into wacc4[:, i, chunk]
    row tail: totals -> 1/sum, normalize weights row + weighted accumulators.
"""

import numpy as np
import ml_dtypes

import concourse.bass as bass
import concourse.tile as tile
from concourse import mybir, bacc
from concourse.bass_utils import run_bass_kernel_spmd

B, S, E, H = 32, 2048, 1024, 1024
NCORES = 8
BS = B // NCORES

F32 = mybir.dt.float32
BF16 = mybir.dt.bfloat16
U8 = mybir.dt.uint8

CH = 512  # matmul moving free-dim chunk (one PSUM bank of f32)


def build(bs=BS, s=S, e=E, h=H, num_devices=NCORES):
    """Build + compile the per-core program. All cores run the same program."""
    P = 128
    s_t, e_t, h_t, g_t = s // P, e // P, h // P, h // P
    n_ch = s // CH          # score chunks per row
    qn = 4 if s_t % 4 == 0 else 1   # s-tiles per cast DMA

    nc = bacc.Bacc("TRN2", target_bir_lowering=False, debug=False,
                   num_devices=num_devices)

    encT_d = nc.dram_tensor("encT", [bs, e, s], BF16, kind="ExternalInput")
    mask_d = nc.dram_tensor("mask", [bs, s], U8, kind="ExternalInput")
    m1t_d = nc.dram_tensor("m1t", [e, h], BF16, kind="ExternalInput")
    s2t_d = nc.dram_tensor("s2t", [P, h // P], F32, kind="ExternalInput")
    hidt_d = nc.dram_tensor("hidt", [h, bs], F32, kind="ExternalInput")
    mbt_d = nc.dram_tensor("mbt", [P, h_t], F32, kind="ExternalInput")
    vwt_d = nc.dram_tensor("vwt", [P, h_t], BF16, kind="ExternalInput")
    weighted_d = nc.dram_tensor("weighted", [bs, e], F32, kind="ExternalOutput")
    weights_d = nc.dram_tensor("weights", [bs, s], F32, kind="ExternalOutput")

    encT_r = encT_d.ap().rearrange("b (i p) s -> p b i s", p=P)
    # weighted row r viewed as [e_lo=128p, e_tile] for the scatter store
    wout_r = weighted_d.ap().rearrange("b (i p) -> p b i", p=P)

    with tile.TileContext(nc) as tc:
        import contextlib
        with contextlib.ExitStack() as ctx:
            consts = ctx.enter_context(tc.tile_pool(name="consts", bufs=1))
            tr_pool = ctx.enter_context(tc.tile_pool(name="encT", bufs=2))
            tanh_pool = ctx.enter_context(tc.tile_pool(name="tanh", bufs=4))
            row1 = ctx.enter_context(tc.tile_pool(name="row1", bufs=1))
            row2 = ctx.enter_context(tc.tile_pool(name="row2", bufs=2))
            ps_big = ctx.enter_context(tc.tile_pool(name="psb", bufs=4, space="PSUM"))
            ps_sc = ctx.enter_context(tc.tile_pool(name="pssc", bufs=4, space="PSUM"))

            # ---- replicated weights into SBUF (scalar HWDGE ring; transposes
            # and per-row small DMAs ride the sync ring) ----
            m1t_sb = consts.tile([P, e_t, h], BF16)
            m1t_r = m1t_d.ap().rearrange("(t p) h -> p t h", p=P)
            for i in range(e_t):
                nc.scalar.dma_start(out=m1t_sb[:, i, :], in_=m1t_r[:, i, :])
            s2t_sb = consts.tile([P, h_t], F32)
            nc.scalar.dma_start(out=s2t_sb, in_=s2t_d.ap())
            hidt_sb = consts.tile([P, g_t, bs], F32)
            nc.scalar.dma_start(out=hidt_sb, in_=hidt_d.ap().rearrange(
                "(t p) b -> p t b", p=P))
            mbt_sb = consts.tile([P, h_t], F32)
            nc.scalar.dma_start(out=mbt_sb, in_=mbt_d.ap())
            vwt_sb = consts.tile([P, h_t], BF16)
            nc.scalar.dma_start(out=vwt_sb, in_=vwt_d.ap())
            shift = consts.tile([1, 1], F32)
            nc.vector.memset(shift, -32.0)

            # ---- hb[h,b] = hidden[b,h]*S2[h]+M_b[h]  (S2 host-precomputed) ----
            hb_sb = consts.tile([P, h_t, bs], F32)
            for j in range(h_t):
                nc.vector.tensor_scalar(
                    out=hb_sb[:, j, :], in0=hidt_sb[:, j, :],
                    scalar1=s2t_sb[:, j:j + 1], scalar2=mbt_sb[:, j:j + 1],
                    op0=mybir.AluOpType.mult, op1=mybir.AluOpType.add)

            # ---- per batch row ----
            def emit_load(r, nchunk):
                """DMA row r of host-transposed bf16 encT into SBUF, by
                s-chunks so chunk 0 is ready early."""
                encT_sb = tr_pool.tile([P, e_t, s], BF16, tag="encT",
                                       name=f"encT{r}")
                cw = s // nchunk
                for q in range(nchunk):
                    nc.sync.dma_start(
                        out=encT_sb[:, :, q * cw:(q + 1) * cw],
                        in_=encT_r[:, r, :, q * cw:(q + 1) * cw])
                return encT_sb

            encTs = {0: emit_load(0, 2 * n_ch)}
            for r in range(bs):
                # prefetch next row's load chain ahead of this row's epilogues
                if r + 1 < bs:
                    encTs[r + 1] = emit_load(r + 1, 2)
                encT = encTs.pop(r)

                # mask row -> mz = 1 - mask (f32), early (off critical path)
                mrow = row2.tile([1, s], U8, tag="mrow", name=f"mrow{r}")
                nc.sync.dma_start(out=mrow, in_=mask_d.ap()[r:r + 1, :])
                mz = row2.tile([1, s], F32, tag="mz", name=f"mz{r}")
                nc.vector.tensor_scalar(
                    out=mz, in0=mrow, scalar1=-1.0, scalar2=1.0,
                    op0=mybir.AluOpType.mult, op1=mybir.AluOpType.add)

                # per-row accumulators
                exps = row2.tile([1, s], F32, tag="exps", name=f"exps{r}")
                expm = row2.tile([1, s], F32, tag="expm", name=f"expm{r}")
                wacc4 = row2.tile([P, e_t, n_ch], F32, tag="wacc4",
                                  name=f"wacc4{r}")
                tot4 = row2.tile([1, n_ch], F32, tag="tot4", name=f"tot4{r}")

                sc_ps = [ps_sc.tile([1, CH], F32, tag="scps", name=f"sc{r}_{c}")
                         for c in range(n_ch)]
                groups = [list(range(i, min(i + 2, n_ch)))
                          for i in range(0, n_ch, 2)]
                for grp in groups:
                    pending = None
                    for j in range(h_t):
                        mt = {}
                        for c in grp:
                            mt[c] = ps_big.tile([P, CH], F32, tag="bigps",
                                                name=f"mt{r}_{j}_{c}")
                        for i in range(e_t):
                            first, last = (i == 0), (i == e_t - 1)
                            for c in grp:
                                nc.tensor.matmul(
                                    out=mt[c],
                                    lhsT=m1t_sb[:, i, j * P:(j + 1) * P],
                                    rhs=encT[:, i, c * CH:(c + 1) * CH],
                                    start=first, stop=last)
                        th = {}
                        for c in grp:
                            th[c] = tanh_pool.tile([P, CH], BF16, tag="th",
                                                   name=f"th{r}_{j}_{c}")
                            nc.scalar.activation(
                                out=th[c], in_=mt[c],
                                func=mybir.ActivationFunctionType.Tanh,
                                bias=hb_sb[:, j, r:r + 1], scale=1.0)
                        if pending is not None:
                            pj, pth = pending
                            for c, t_sb in pth.items():
                                nc.tensor.matmul(
                                    out=sc_ps[c], lhsT=vwt_sb[:, pj:pj + 1],
                                    rhs=t_sb,
                                    start=(pj == 0), stop=(pj == h_t - 1))
                        pending = (j, th)
                    pj, pth = pending
                    for c, t_sb in pth.items():
                        nc.tensor.matmul(
                            out=sc_ps[c], lhsT=vwt_sb[:, pj:pj + 1], rhs=t_sb,
                            start=(pj == 0), stop=(pj == h_t - 1))

                    # ---- chunk epilogue: exp, mask, broadcast, mul+reduce ----
                    for c in grp:
                        cs = slice(c * CH, (c + 1) * CH)
                        nc.scalar.activation(
                            out=exps[:, cs], in_=sc_ps[c],
                            func=mybir.ActivationFunctionType.Exp,
                            bias=shift[0:1, 0:1], scale=1.0)
                        nc.vector.tensor_mul(expm[:, cs], exps[:, cs], mz[:, cs])
                        nc.vector.tensor_reduce(
                            out=tot4[:, c:c + 1], in_=expm[:, cs],
                            axis=mybir.AxisListType.X, op=mybir.AluOpType.add)
                        wbf = row2.tile([1, CH], BF16, tag="wbf",
                                        name=f"wbf{r}_{c}")
                        nc.vector.tensor_copy(out=wbf, in_=expm[:, cs])
                        wbc = row2.tile([P, CH], BF16, tag="wbc",
                                        name=f"wbc{r}_{c}")
                        nc.gpsimd.partition_broadcast(wbc, wbf)
                        for i in range(e_t):
                            prod = row2.tile([P, CH], BF16, tag="prod",
                                             name=f"prod{r}_{c}_{i}")
                            nc.vector.tensor_mul(prod, encT[:, i, cs], wbc)
                            nc.vector.tensor_reduce(
                                out=wacc4[:, i, c:c + 1], in_=prod,
                                axis=mybir.AxisListType.X,
                                op=mybir.AluOpType.add)

                # ---- row tail: totals, normalize, outputs ----
                tot = row1.tile([1, 1], F32, tag="tot", name=f"tot{r}")
                nc.vector.tensor_reduce(
                    out=tot, in_=tot4, axis=mybir.AxisListType.X,
                    op=mybir.AluOpType.add)
                inv = row1.tile([1, 1], F32, tag="inv", name=f"inv{r}")
                nc.vector.reciprocal(inv, tot)

                wrow = row2.tile([1, s], F32, tag="wrow", name=f"wrow{r}")
                nc.vector.tensor_scalar_mul(wrow, expm, inv[0:1, 0:1])
                nc.sync.dma_start(out=weights_d.ap()[r:r + 1, :], in_=wrow)

                invb = row1.tile([P, 1], F32, tag="invb", name=f"invb{r}")
                nc.gpsimd.partition_broadcast(invb, inv)
                wacc = row1.tile([P, e_t], F32, tag="wacc", name=f"wacc{r}")
                nc.vector.tensor_reduce(
                    out=wacc, in_=wacc4, axis=mybir.AxisListType.X,
                    op=mybir.AluOpType.add)
                wfin = row1.tile([P, e_t], F32, tag="wfin", name=f"wfin{r}")
                nc.vector.tensor_scalar_mul(wfin, wacc, invb[:, 0:1])
                nc.sync.dma_start(out=wout_r[:, r, :], in_=wfin)

    nc.compile()
    return nc


_NC_CACHE = {}


def _get_nc():
    if "nc" not in _NC_CACHE:
        _NC_CACHE["nc"] = build()
    return _NC_CACHE["nc"]


def kernel(hidden, encoder_outputs, mask, M_w, M_b, V_w, V_b,
           _trace=False, _return_res=False):
    P = 128
    h_t = H // P
    hidden = np.asarray(hidden, dtype=np.float32)
    enc = np.asarray(encoder_outputs, dtype=np.float32)
    mask_u8 = np.asarray(mask).astype(np.uint8)
    M_w = np.asarray(M_w, dtype=np.float32)
    M_b = np.asarray(M_b, dtype=np.float32)
    V_w = np.asarray(V_w, dtype=np.float32)

    nc = _get_nc()

    m1t = np.ascontiguousarray(M_w[:, :E].T).astype(ml_dtypes.bfloat16)
    s2 = M_w[:, E:].astype(np.float64).sum(axis=0).astype(np.float32)
    s2t = np.ascontiguousarray(s2.reshape(h_t, P).T)
    mbt = np.ascontiguousarray(M_b.reshape(h_t, P).T).astype(np.float32)
    vwt = np.ascontiguousarray(V_w[0].reshape(h_t, P).T).astype(ml_dtypes.bfloat16)

    in_maps = []
    for core in range(NCORES):
        rows = slice(core * BS, (core + 1) * BS)
        hidt = np.ascontiguousarray(hidden[rows, 0, :].T).astype(np.float32)
        encT = np.ascontiguousarray(
            enc[rows].transpose(0, 2, 1)).astype(ml_dtypes.bfloat16)
        in_maps.append({
            "encT": encT,
            "mask": np.ascontiguousarray(mask_u8[rows]),
            "m1t": m1t, "s2t": s2t, "hidt": hidt, "mbt": mbt, "vwt": vwt,
        })

    res = run_bass_kernel_spmd(nc, in_maps, list(range(NCORES)), trace=_trace)

    weighted = np.concatenate(
        [res.results[c]["weighted"] for c in range(NCORES)], axis=0)
    weights = np.concatenate(
        [res.results[c]["weights"] for c in range(NCORES)], axis=0)
    out = (weighted[:, None, :].astype(np.float32),
           weights[:, None, :].astype(np.float32))
    if _return_res:
        return out, res
    return out


# revision 17
# speedup vs baseline: 1.6359x; 1.0365x over previous
"""Bass/Tile TRN2 kernel for nn_Attn_52776558133289 (additive attention).

Reference math (per batch row b) — NOTE the reference's hidden-term einsum
'boh,hg->boh' is elementwise-times-column-sums, NOT a matvec:
    m[s, h]   = enc[b] @ M_w[:, :E].T + hidden[b,0,h]*S2[h] + M_b[h]
                with S2[h] = sum_g M_w[g, E+h]
    scores[s] = tanh(m) @ V_w[0] + V_b          (V_b cancels in softmax)
    scores    = where(mask, -1e30, scores)
    weights   = softmax(scores)                  (B, S)
    weighted  = weights @ enc[b]                 (B, E)

Sharding: data-parallel over B across 8 NeuronCores; M/V weights replicated.

Per-core dataflow (b_shard rows each):
    encT (host-prepped transposed bf16 copy of enc) --DMA--> SBUF
    encT layout: [e_lo=128p, e_tile, s]
    m.T tile = matmul(lhsT=M1T[e,h-tile], rhs=encT chunk)   (PSUM f32)
    tanh     = ACT Tanh(m.T + hb[h])  -> bf16   (hb = hidden*S2 + M_b, on device)
    scores  += matmul(lhsT=VwT[:,j], rhs=tanh)              (1 x 512 PSUM)
    per s-chunk epilogue (overlaps later matmuls):
      exp(s-32) on ACT (|s| <= H*max|Vw| = 32 exactly => fixed shift, no max),
      mask-zero, bf16 cast, partition_broadcast, then per e-tile
      DVE mul+reduce into wacc4[:, i, chunk]
    row tail: totals -> 1/sum, normalize weights row + weighted accumulators.
"""

import numpy as np
import ml_dtypes

import concourse.bass as bass
import concourse.tile as tile
from concourse import mybir, bacc
from concourse.bass_utils import run_bass_kernel_spmd

B, S, E, H = 32, 2048, 1024, 1024
NCORES = 8
BS = B // NCORES

F32 = mybir.dt.float32
BF16 = mybir.dt.bfloat16
U8 = mybir.dt.uint8

CH = 512  # matmul moving free-dim chunk (one PSUM bank of f32)


def build(bs=BS, s=S, e=E, h=H, num_devices=NCORES):
    """Build + compile the per-core program. All cores run the same program."""
    P = 128
    s_t, e_t, h_t, g_t = s // P, e // P, h // P, h // P
    n_ch = s // CH          # score chunks per row
    qn = 4 if s_t % 4 == 0 else 1   # s-tiles per cast DMA

    nc = bacc.Bacc("TRN2", target_bir_lowering=False, debug=False,
                   num_devices=num_devices)

    encT_d = nc.dram_tensor("encT", [bs, e, s], BF16, kind="ExternalInput")
    mask_d = nc.dram_tensor("mask", [bs, s], U8, kind="ExternalInput")
    m1t_d = nc.dram_tensor("m1t", [e, h], BF16, kind="ExternalInput")
    s2t_d = nc.dram_tensor("s2t", [P, h // P], F32, kind="ExternalInput")
    hidt_d = nc.dram_tensor("hidt", [h, bs], F32, kind="ExternalInput")
    mbt_d = nc.dram_tensor("mbt", [P, h_t], F32, kind="ExternalInput")
    vwt_d = nc.dram_tensor("vwt", [P, h_t], BF16, kind="ExternalInput")
    weighted_d = nc.dram_tensor("weighted", [bs, e], F32, kind="ExternalOutput")
    weights_d = nc.dram_tensor("weights", [bs, s], F32, kind="ExternalOutput")

    encT_r = encT_d.ap().rearrange("b (i p) s -> p b i s", p=P)
    # weighted row r viewed as [e_lo=128p, e_tile] for the scatter store
    wout_r = weighted_d.ap().rearrange("b (i p) -> p b i", p=P)

    with tile.TileContext(nc) as tc:
        import contextlib
        with contextlib.ExitStack() as ctx:
            consts = ctx.enter_context(tc.tile_pool(name="consts", bufs=1))
            tr_pool = ctx.enter_context(tc.tile_pool(name="encT", bufs=2))
            tanh_pool = ctx.enter_context(tc.tile_pool(name="tanh", bufs=4))
            row1 = ctx.enter_context(tc.tile_pool(name="row1", bufs=1))
            row2 = ctx.enter_context(tc.tile_pool(name="row2", bufs=2))
            ps_big = ctx.enter_context(tc.tile_pool(name="psb", bufs=4, space="PSUM"))
            ps_sc = ctx.enter_context(tc.tile_pool(name="pssc", bufs=4, space="PSUM"))

            # ---- replicated weights into SBUF (scalar HWDGE ring; transposes
            # and per-row small DMAs ride the sync ring) ----
            m1t_sb = consts.tile([P, e_t, h], BF16)
            m1t_r = m1t_d.ap().rearrange("(t p) h -> p t h", p=P)
            for i in range(e_t):
                nc.scalar.dma_start(out=m1t_sb[:, i, :], in_=m1t_r[:, i, :])
            s2t_sb = consts.tile([P, h_t], F32)
            nc.scalar.dma_start(out=s2t_sb, in_=s2t_d.ap())
            hidt_sb = consts.tile([P, g_t, bs], F32)
            nc.scalar.dma_start(out=hidt_sb, in_=hidt_d.ap().rearrange(
                "(t p) b -> p t b", p=P))
            mbt_sb = consts.tile([P, h_t], F32)
            nc.scalar.dma_start(out=mbt_sb, in_=mbt_d.ap())
            vwt_sb = consts.tile([P, h_t], BF16)
            nc.scalar.dma_start(out=vwt_sb, in_=vwt_d.ap())
            shift = consts.tile([1, 1], F32)
            nc.vector.memset(shift, -32.0)

            # ---- hb[h,b] = hidden[b,h]*S2[h]+M_b[h]  (S2 host-precomputed) ----
            hb_sb = consts.tile([P, h_t, bs], F32)
            for j in range(h_t):
                nc.vector.tensor_scalar(
                    out=hb_sb[:, j, :], in0=hidt_sb[:, j, :],
                    scalar1=s2t_sb[:, j:j + 1], scalar2=mbt_sb[:, j:j + 1],
                    op0=mybir.AluOpType.mult, op1=mybir.AluOpType.add)

            # ---- per batch row ----
            def emit_load(r, nchunk):
                """DMA row r of host-transposed bf16 encT into SBUF, by
                s-chunks so chunk 0 is ready early."""
                encT_sb = tr_pool.tile([P, e_t, s], BF16, tag="encT",
                                       name=f"encT{r}")
                cw = s // nchunk
                for q in range(nchunk):
                    nc.sync.dma_start(
                        out=encT_sb[:, :, q * cw:(q + 1) * cw],
                        in_=encT_r[:, r, :, q * cw:(q + 1) * cw])
                return encT_sb

            encTs = {0: emit_load(0, 2 * n_ch)}
            for r in range(bs):
                # prefetch next row's load chain ahead of this row's epilogues
                if r + 1 < bs:
                    encTs[r + 1] = emit_load(r + 1, 2)
                encT = encTs.pop(r)

                # mask row -> mz = 1 - mask (f32), early (off critical path)
                mrow = row2.tile([1, s], U8, tag="mrow", name=f"mrow{r}")
                nc.sync.dma_start(out=mrow, in_=mask_d.ap()[r:r + 1, :])
                mz = row2.tile([1, s], F32, tag="mz", name=f"mz{r}")
                nc.vector.tensor_scalar(
                    out=mz, in0=mrow, scalar1=-1.0, scalar2=1.0,
                    op0=mybir.AluOpType.mult, op1=mybir.AluOpType.add)

                # per-row accumulators
                exps = row2.tile([1, s], F32, tag="exps", name=f"exps{r}")
                expm = row2.tile([1, s], F32, tag="expm", name=f"expm{r}")
                wacc4 = row2.tile([P, e_t, n_ch], F32, tag="wacc4",
                                  name=f"wacc4{r}")
                tot4 = row2.tile([1, n_ch], F32, tag="tot4", name=f"tot4{r}")

                sc_ps = [ps_sc.tile([1, CH], F32, tag="scps", name=f"sc{r}_{c}")
                         for c in range(n_ch)]
                if r == bs - 1 and n_ch == 4:
                    # last row: tiny final group so the exposed tail epilogue
                    # (softmax+weighted for the last chunk) is minimal
                    groups = [[0, 1], [2], [3]]
                else:
                    groups = [list(range(i, min(i + 2, n_ch)))
                              for i in range(0, n_ch, 2)]
                for grp in groups:
                    pending = None
                    for j in range(h_t):
                        mt = {}
                        for c in grp:
                            mt[c] = ps_big.tile([P, CH], F32, tag="bigps",
                                                name=f"mt{r}_{j}_{c}")
                        for i in range(e_t):
                            first, last = (i == 0), (i == e_t - 1)
                            for c in grp:
                                nc.tensor.matmul(
                                    out=mt[c],
                                    lhsT=m1t_sb[:, i, j * P:(j + 1) * P],
                                    rhs=encT[:, i, c * CH:(c + 1) * CH],
                                    start=first, stop=last)
                        th = {}
                        for c in grp:
                            th[c] = tanh_pool.tile([P, CH], BF16, tag="th",
                                                   name=f"th{r}_{j}_{c}")
                            nc.scalar.activation(
                                out=th[c], in_=mt[c],
                                func=mybir.ActivationFunctionType.Tanh,
                                bias=hb_sb[:, j, r:r + 1], scale=1.0)
                        if pending is not None:
                            pj, pth = pending
                            for c, t_sb in pth.items():
                                nc.tensor.matmul(
                                    out=sc_ps[c], lhsT=vwt_sb[:, pj:pj + 1],
                                    rhs=t_sb,
                                    start=(pj == 0), stop=(pj == h_t - 1))
                        pending = (j, th)
                    pj, pth = pending
                    for c, t_sb in pth.items():
                        nc.tensor.matmul(
                            out=sc_ps[c], lhsT=vwt_sb[:, pj:pj + 1], rhs=t_sb,
                            start=(pj == 0), stop=(pj == h_t - 1))

                    # ---- chunk epilogue: exp, mask, broadcast, mul+reduce ----
                    for c in grp:
                        cs = slice(c * CH, (c + 1) * CH)
                        nc.scalar.activation(
                            out=exps[:, cs], in_=sc_ps[c],
                            func=mybir.ActivationFunctionType.Exp,
                            bias=shift[0:1, 0:1], scale=1.0)
                        nc.vector.tensor_mul(expm[:, cs], exps[:, cs], mz[:, cs])
                        nc.vector.tensor_reduce(
                            out=tot4[:, c:c + 1], in_=expm[:, cs],
                            axis=mybir.AxisListType.X, op=mybir.AluOpType.add)
                        wbf = row2.tile([1, CH], BF16, tag="wbf",
                                        name=f"wbf{r}_{c}")
                        nc.vector.tensor_copy(out=wbf, in_=expm[:, cs])
                        wbc = row2.tile([P, CH], BF16, tag="wbc",
                                        name=f"wbc{r}_{c}")
                        nc.gpsimd.partition_broadcast(wbc, wbf)
                        for i in range(e_t):
                            prod = row2.tile([P, CH], BF16, tag="prod",
                                             name=f"prod{r}_{c}_{i}")
                            nc.vector.tensor_mul(prod, encT[:, i, cs], wbc)
                            nc.vector.tensor_reduce(
                                out=wacc4[:, i, c:c + 1], in_=prod,
                                axis=mybir.AxisListType.X,
                                op=mybir.AluOpType.add)

                # ---- row tail: totals, normalize, outputs ----
                tot = row1.tile([1, 1], F32, tag="tot", name=f"tot{r}")
                nc.vector.tensor_reduce(
                    out=tot, in_=tot4, axis=mybir.AxisListType.X,
                    op=mybir.AluOpType.add)
                inv = row1.tile([1, 1], F32, tag="inv", name=f"inv{r}")
                nc.vector.reciprocal(inv, tot)

                wrow = row2.tile([1, s], F32, tag="wrow", name=f"wrow{r}")
                nc.vector.tensor_scalar_mul(wrow, expm, inv[0:1, 0:1])
                nc.sync.dma_start(out=weights_d.ap()[r:r + 1, :], in_=wrow)

                invb = row1.tile([P, 1], F32, tag="invb", name=f"invb{r}")
                nc.gpsimd.partition_broadcast(invb, inv)
                wacc = row1.tile([P, e_t], F32, tag="wacc", name=f"wacc{r}")
                nc.vector.tensor_reduce(
                    out=wacc, in_=wacc4, axis=mybir.AxisListType.X,
                    op=mybir.AluOpType.add)
                wfin = row1.tile([P, e_t], F32, tag="wfin", name=f"wfin{r}")
                nc.vector.tensor_scalar_mul(wfin, wacc, invb[:, 0:1])
                nc.sync.dma_start(out=wout_r[:, r, :], in_=wfin)

    nc.compile()
    return nc


_NC_CACHE = {}


def _get_nc():
    if "nc" not in _NC_CACHE:
        _NC_CACHE["nc"] = build()
    return _NC_CACHE["nc"]


def kernel(hidden, encoder_outputs, mask, M_w, M_b, V_w, V_b,
           _trace=False, _return_res=False):
    P = 128
    h_t = H // P
    hidden = np.asarray(hidden, dtype=np.float32)
    enc = np.asarray(encoder_outputs, dtype=np.float32)
    mask_u8 = np.asarray(mask).astype(np.uint8)
    M_w = np.asarray(M_w, dtype=np.float32)
    M_b = np.asarray(M_b, dtype=np.float32)
    V_w = np.asarray(V_w, dtype=np.float32)

    nc = _get_nc()

    m1t = np.ascontiguousarray(M_w[:, :E].T).astype(ml_dtypes.bfloat16)
    s2 = M_w[:, E:].astype(np.float64).sum(axis=0).astype(np.float32)
    s2t = np.ascontiguousarray(s2.reshape(h_t, P).T)
    mbt = np.ascontiguousarray(M_b.reshape(h_t, P).T).astype(np.float32)
    vwt = np.ascontiguousarray(V_w[0].reshape(h_t, P).T).astype(ml_dtypes.bfloat16)

    in_maps = []
    for core in range(NCORES):
        rows = slice(core * BS, (core + 1) * BS)
        hidt = np.ascontiguousarray(hidden[rows, 0, :].T).astype(np.float32)
        encT = np.ascontiguousarray(
            enc[rows].transpose(0, 2, 1)).astype(ml_dtypes.bfloat16)
        in_maps.append({
            "encT": encT,
            "mask": np.ascontiguousarray(mask_u8[rows]),
            "m1t": m1t, "s2t": s2t, "hidt": hidt, "mbt": mbt, "vwt": vwt,
        })

    res = run_bass_kernel_spmd(nc, in_maps, list(range(NCORES)), trace=_trace)

    weighted = np.concatenate(
        [res.results[c]["weighted"] for c in range(NCORES)], axis=0)
    weights = np.concatenate(
        [res.results[c]["weights"] for c in range(NCORES)], axis=0)
    out = (weighted[:, None, :].astype(np.float32),
           weights[:, None, :].astype(np.float32))
    if _return_res:
        return out, res
    return out
